# revision 62
# baseline (speedup 1.0000x reference)
"""Trainium2 Bass kernel for BSplineNN: cubic B-spline evaluation.

out[b, c] = sum_i coefficients[b, i, c] * N_{i,3}(x_b),  x_b = inpce[b, 0]

Per core (512 batches, layout b = g*128 + p): the host packs each
coefficient row with its knot window (combo row i = [coef[b, i, 0:256] |
knots[b, i:i+8]], 264 f32 = 1056B; value-independent layout marshalling).
On device: compute the knot interval i0 = #{t[4:64] <= x} on DVE, then 4
single-index-per-partition SWDGE indirect gathers pull rows i0..i0+3 per
batch -- row i0 carries the Cox-de Boor window t[i0..i0+7] for free, which
eliminated the 4 separate knot-window gathers (-1.35us, 13.2 -> 11.8us).
Merged Cox-de Boor on DVE over all 4 groups at once (batched divided
differences via overlapping strided APs into the gathered tile), weighted
sum split DVE/ACT (act_groups=2), one merged output store.

Perf notes (all HW-measured, per-iteration steady state):
  - v2 baseline (8 gathers): 13.1-13.6us. combo rows: 11.8us.
  - The regime is op-dispatch/serialization-bound, not bandwidth-bound:
    bf16 coefficients (halves gathered bytes) measured SLOWER (14.3us),
    act_groups=4 (all-ACT wsum) 17.3us, 4 SWDGE queues + 64KB ring 16.0us,
    bufs=5 13.3us, single_packet=False 47us.
  - dma_gather (InstDMAGatherAnt, 512 idx x 4KB) = ~30us: the extended-inst
    Q7 path costs ~58ns/descriptor vs native SWDGE 0.34ns/desc. Dead end
    (kept behind use_dma_gather/use_kwin for reference; that path is
    otherwise correct: it needs indices int16-wrapped [i%16, i//16] AND
    replicated to all eight 16-partition groups because the ucode reads
    them from partitions [16+32q, 32+32q) for queue q -- done via a DRAM
    round-trip with a rearranged store AP).
  - multi-index indirect_dma_start (>1 index/partition) returns wrong data
    for every group on HW (multiidx_test.py), so the remaining 4 gathers
    cannot be merged further.

Sharding: pure data parallel, batch dim split across 8 cores (512 each).
"""

import numpy as np

import concourse.bacc as bacc
import concourse.bass as bass
import concourse.mybir as mybir
import concourse.tile as tile
from concourse import library_config
from concourse.bass_utils import run_bass_kernel_spmd

B, N, C, T = 4096, 64, 256, 68   # batch, coef rows, channels, knots
K = 3                            # cubic
NCORES = 8
BC = B // NCORES                 # 512 batches per core
P = 128                          # partitions
G = BC // P                      # 4 batch-groups per partition
WROWS = K + 1                    # 4 gathered coef rows per batch
WKNOTS = 2 * K + 2               # 8 gathered knots per batch
KWPAD = G * WKNOTS + 4           # kw tile padded for overlapping D reads
CELEM = WROWS * C                # 1024 floats per gathered coef vector
CSTEP = C                        # gather row granularity: one coef row
NROWS = (BC * N * C - CELEM) // CSTEP + 1  # 32765 valid gather row starts
NWIN = N - WROWS + 1             # 61 window phases (i0 in [0, 60])
WELEM = 64                       # floats per kwin row (256B gather minimum)
F32 = mybir.dt.float32
I32 = mybir.dt.int32
I16 = mybir.dt.int16

# NOTE (measured on HW this session):
#  - dma_gather (InstDMAGatherAnt) is ~30us for 512x4KB -- the extended-inst
#    Q7 path costs ~58ns/descriptor, far above the native SWDGE 0.34ns/desc.
#    use_dma_gather/use_kwin are kept only as documentation of that dead end.
#  - multi-index indirect_dma_start (>1 index per partition) returns wrong
#    data for every group (multiidx_test.py), so the 8 per-group single-index
#    indirect gathers cannot be merged.
DEFAULTS = dict(
    act_groups=2,             # trailing groups whose wsum mults go on ACT
    batched_d=True,           # batched overlapping-AP D/U (else per-kk slices)
    use_dma_gather=False,     # one dma_gather for coef (else 4x indirect)
    use_kwin=False,           # knot windows via dma_gather over host-built
                              # sliding-window tensor (else 4x indirect)
    single_packet=True,       # (dma_gather only) concat descs into 1 packet
    sp_indirect=False,        # single_packet on the indirect coef gathers
    nq=1,                     # SWDGE queues; indirect gathers round-robin
    coef_bf16=False,          # host-cast coefficients to bf16: halves the
                              # gathered HBM bytes (rel err ~4e-3 < 2e-2 gate)
    combo=True,               # host-pack [256 f32 coef | 8 f32 knots] per
                              # row: the coef gathers carry the knot windows,
                              # eliminating the 4 knot-window SWDGE gathers
    ktx=False,                # host-pack [knots[4:64] | x] (61 f32/batch):
                              # merges the kt + xt loads into one DMA
    debug_dump=None,          # 'i0' | 'wts' | 'gt': dump to out
)
BF16 = mybir.dt.bfloat16
CROW = C + WKNOTS             # 264 f32 elements per combo row (1056B)


def _strided(a, dims, extra_offset):
    """Overlapping strided free-axis view of a 2-D [P, F] AP.

    dims: list of [stride, count] free dims (innermost last)."""
    b = a.copy()
    V = type(b.ap)
    b.ap = V([list(a.ap[0])] + [list(d) for d in dims])
    b.offset = a.offset + extra_offset
    return b


def _gather_src(coef_ap):
    """Overlapping rows view for dma_gather: row r = coef_flat[256r : 256r+1024]."""
    flat = coef_ap.rearrange("b n c -> (b n c)")
    g = flat.copy()
    V = type(g.ap)
    g.ap = V([[CSTEP, NROWS], [1, CELEM]])
    return g


def _emit_hoisted(tc, nc, hp, use_kwin=True, use_dma_gather=False):
    """Loop-invariant index bases per (p, g); local batch b = g*128 + p."""
    bi = hp.tile([P, G], I32, tag="bi")
    nc.gpsimd.iota(out=bi[:], pattern=[[P * N, G]], base=0,
                   channel_multiplier=N)
    kmul = NWIN if use_kwin else T
    bik = hp.tile([P, G], I32, tag="bik")
    nc.gpsimd.iota(out=bik[:], pattern=[[P * kmul, G]], base=0,
                   channel_multiplier=kmul)
    if use_dma_gather:
        nc.gpsimd.load_library(library_config.mlp)
    return bi, bik


def _q(ins, g, nq, sp=False):
    """Round-robin an indirect DMA onto qPoolDynamic{0..nq-1}; optionally
    concatenate each engine's descriptors into one packet (single_packet)."""
    if nq > 1:
        q = g % nq
        ins.ins.queue = f"qPoolDynamic{q or ''}"
    if sp:
        ins.ins.single_packet = True
    return ins


def _emit(tc, nc, sb, hoisted, coef, knots, inpce, out, kwin=None,
          act_groups=0, batched_d=True, use_dma_gather=True, use_kwin=True,
          single_packet=True, sp_indirect=False, nq=1, coef_bf16=False,
          combo=False, ktx=False, debug_dump=None):
    bi, bik = hoisted
    assert not use_kwin or (use_dma_gather and kwin is not None)

    # ---- load the 60 middle knots (all i0 needs) + x (layout b = g*128+p) --
    NM = N - WROWS  # 60
    if ktx:
        # knots is the host-packed ktx tensor [BC, 61]: cols 0:60 are
        # knots[4:64], col 60 is x. One DMA replaces the kt + xt loads.
        ktt = sb.tile([P, G, NM + 1], F32, tag="ktt")
        nc.sync.dma_start(
            out=ktt[:], in_=knots.rearrange("(g p) t -> p g t", p=P))
        ktv = ktt[:][:, :, 0:NM]
        xt_ap = ktt[:][:, :, NM:NM + 1].rearrange("p g o -> p (g o)")
    else:
        kt = sb.tile([P, G, NM], F32, tag="kt")
        nc.sync.dma_start(
            out=kt[:],
            in_=knots.rearrange("(g p) t -> p g t", p=P)[:, :, WROWS:N])
        xt = sb.tile([P, G], F32, tag="xt")
        nc.scalar.dma_start(out=xt[:],
                            in_=inpce.rearrange("(g p) o -> p (g o)", p=P))
        ktv = kt[:]
        xt_ap = xt[:]

    # ---- interval index: i0 = #{j in [4,64): t[j] <= x} in [0, 60] ----
    ind = sb.tile([P, G, NM], F32, tag="ind")
    nc.vector.tensor_tensor(out=ind[:],
                            in0=xt_ap.to_broadcast([P, G, NM]),
                            in1=ktv,
                            op=mybir.AluOpType.is_ge)
    i0f = sb.tile([P, G], F32, tag="i0f")
    nc.vector.reduce_sum(out=i0f[:], in_=ind[:], axis=mybir.AxisListType.X)
    i0i = sb.tile([P, G], I32, tag="i0i")
    nc.vector.tensor_copy(out=i0i[:], in_=i0f[:])

    # ---- gathers ----
    gt = sb.tile([P, G, WROWS * (CROW if combo else C)],
                 BF16 if coef_bf16 else F32, tag="gt")
    if use_dma_gather:
        # index tiles, int16, packed [cidx | kidx] along the free axis
        H = 2 if use_kwin else 1
        st32 = sb.tile([P, H, G], I32, tag="st32")
        nc.vector.tensor_tensor(out=st32[:][:, 0, :], in0=bi[:], in1=i0i[:],
                                op=mybir.AluOpType.add)
        if use_kwin:
            nc.vector.tensor_tensor(out=st32[:][:, 1, :], in0=bik[:],
                                    in1=i0i[:], op=mybir.AluOpType.add)
        st16 = sb.tile([P, H * G], I16, tag="st16")
        nc.vector.tensor_copy(out=st16[:],
                              in_=st32[:].rearrange("p h g -> p (h g)"))
        # cross-partition rewrap via DRAM: batch b = g*128 + (16w+q) goes to
        # scratch[q, h, 8g+w]; in iteration [p][h][g] == out [w][q][h][g].
        scratch = sb.tile([16, H * G * 8], I16, tag="cscr", space="DRAM")
        nc.sync.dma_start(
            out=scratch[:].rearrange("q (h g w) -> w q h g", h=H, w=8),
            in_=st16[:].rearrange("p (h g) -> p h g", h=H))
        # The dma_gather ucode reads its indices from partition window
        # [16+32q, 32+32q) for queue_num q (set_dtype_channels in
        # dma_gather.cpp), so the [16, .] pattern must be replicated to all
        # eight 16-partition groups: stride-0 leading dim on the DRAM side.
        idxt = sb.tile([P, H * G * 8], I16, tag="idxt")
        rep = scratch[:].copy()
        V = type(rep.ap)
        rep.ap = V([[0, P // 16]] + [list(d) for d in scratch[:].ap])
        nc.sync.dma_start(out=idxt[:], in_=rep)
        if debug_dump == "idxt":
            idf = sb.tile([P, H * G * 8], F32, tag="idf")
            nc.vector.tensor_copy(out=idf[:], in_=idxt[:])
            nc.sync.dma_start(
                out=out.rearrange("(g p) c -> p g c", p=P)[:, 0,
                                                           0:H * G * 8],
                in_=idf[:])
            return
        nc.gpsimd.dma_gather(
            out_ap=gt[:], in_ap=_gather_src(coef),
            idxs_ap=idxt[:][:, 0:G * 8],
            num_idxs=BC, num_idxs_reg=BC,
            elem_size=CELEM, elem_step=CSTEP, queue_num=0,
            single_packet=single_packet)
        if use_kwin:
            kn = sb.tile([P, G, WELEM], F32, tag="kn")
            nc.gpsimd.dma_gather(
                out_ap=kn[:], in_ap=kwin.rearrange("b m e -> (b m) e"),
                idxs_ap=idxt[:][:, G * 8:2 * G * 8],
                num_idxs=BC, num_idxs_reg=BC,
                elem_size=WELEM, queue_num=0,
                single_packet=single_packet)
    else:
        gidx = sb.tile([P, G], I32, tag="gidx")
        nc.vector.tensor_tensor(out=gidx[:], in0=bi[:], in1=i0i[:],
                                op=mybir.AluOpType.add)
        for g in range(G):
            _q(nc.gpsimd.indirect_dma_start(
                out=gt[:][:, g, :], out_offset=None,
                in_=coef.rearrange("b n c -> (b n) c"),
                in_offset=bass.IndirectOffsetOnAxis(ap=gidx[:][:, g:g + 1],
                                                    axis=0)), g, nq,
               sp=sp_indirect)

    if combo:
        # knot windows ride along in the gathered combo rows: row 0 of each
        # batch's 4 rows holds t[i0..i0+7] at f32 offset C.
        gfl = gt[:].rearrange("p g e -> p (g e)")
        gs = WROWS * CROW  # 1056 f32 elements between group windows
        kwf = gfl
        kwv = _strided(gfl, [[gs, G], [1, WKNOTS]], C)
        kwf_off = C
    elif use_kwin:
        gs = WELEM
        kwf = kn[:].rearrange("p g e -> p (g e)")  # [P, G*64]
        kwv = kn[:][:, :, 0:WKNOTS]
        kwf_off = 0
    else:
        # knot-window gathers: 4x 1-idx-per-partition indirect SWDGE
        kidx = sb.tile([P, G], I32, tag="kidx")
        nc.vector.tensor_tensor(out=kidx[:], in0=bik[:], in1=i0i[:],
                                op=mybir.AluOpType.add)
        gs = WKNOTS
        kw = sb.tile([P, KWPAD], F32, tag="kw")
        nc.gpsimd.memset(kw[:][:, G * WKNOTS:KWPAD], 0.0)
        kwf = kw[:]
        kwv = kw[:][:, 0:G * WKNOTS].rearrange("p (g w) -> p g w", g=G)
        kwf_off = 0
        for g in range(G):
            _q(nc.gpsimd.indirect_dma_start(
                out=kwv[:, g, :], out_offset=None,
                in_=knots.rearrange("b (t o) -> (b t) o", o=1),
                in_offset=bass.IndirectOffsetOnAxis(ap=kidx[:][:, g:g + 1],
                                                    axis=0)), g, nq)

    # ---- merged Cox-de Boor over all G groups ----
    # xmt[j] = x - t[i0+j]; ind8[j] = (x >= t[i0+j]); B0[j] = ind8[j]-ind8[j+1]
    xb8 = xt_ap.to_broadcast([P, G, WKNOTS])
    xmt = sb.tile([P, G, WKNOTS], F32, tag="xmt")
    nc.vector.tensor_tensor(out=xmt[:], in0=xb8, in1=kwv,
                            op=mybir.AluOpType.subtract)
    ind8 = sb.tile([P, G, WKNOTS], F32, tag="ind8")
    nc.vector.tensor_tensor(out=ind8[:], in0=xb8, in1=kwv,
                            op=mybir.AluOpType.is_ge)
    B0 = sb.tile([P, G, WKNOTS - 1], F32, tag="B0")
    nc.vector.tensor_tensor(out=B0[:], in0=ind8[:][:, :, 0:WKNOTS - 1],
                            in1=ind8[:][:, :, 1:WKNOTS],
                            op=mybir.AluOpType.subtract)

    # batched divided differences D[g, kk, j] = t[g, j+kk+1] - t[g, j],
    # kk = 0..2 (level kk+1), j = 0..6. hi reads kw-source flat offsets
    # kwf_off + g*gs + kk + 1 + j; overreads beyond the 8-float window stay
    # inside the source tile (pad/next-row garbage) and those lanes of D/R/U
    # are never consumed by the level ops.
    W1 = WKNOTS - 1
    D = sb.tile([P, G, K, W1], F32, tag="D")
    U = sb.tile([P, G, K, W1], F32, tag="U")
    R = sb.tile([P, G, K, W1], F32, tag="R")
    assert batched_d
    hi = _strided(kwf, [[gs, G], [1, K], [1, W1]], kwf_off + 1)
    lo = _strided(kwf, [[gs, G], [0, K], [1, W1]], kwf_off)
    nc.vector.tensor_tensor(out=D[:], in0=hi, in1=lo,
                            op=mybir.AluOpType.subtract)
    nc.vector.reciprocal(out=R[:].rearrange("p g k w -> p (g k w)"),
                         in_=D[:].rearrange("p g k w -> p (g k w)"))
    # U[g, kk, j] = xmt[g, j] * R[g, kk, j]
    xmt_b = _strided(xmt[:], [[WKNOTS, G], [0, K], [1, W1]], 0)
    nc.vector.tensor_tensor(out=U[:], in0=xmt_b, in1=R[:],
                            op=mybir.AluOpType.mult)

    # levels: Bk[i] = a[i] + (B[i+1] - a[i+1]),  a = U[kk-1] .* B (width L+1)
    prev = B0
    for kk in range(1, K + 1):
        L = WKNOTS - 1 - kk
        a = sb.tile([P, G, L + 1], F32, tag=f"a{kk}")
        nc.vector.tensor_tensor(out=a[:], in0=U[:][:, :, kk - 1, 0:L + 1],
                                in1=prev[:][:, :, 0:L + 1],
                                op=mybir.AluOpType.mult)
        t2 = sb.tile([P, G, L], F32, tag=f"t2{kk}")
        nc.vector.tensor_tensor(out=t2[:], in0=prev[:][:, :, 1:L + 1],
                                in1=a[:][:, :, 1:L + 1],
                                op=mybir.AluOpType.subtract)
        nxt = sb.tile([P, G, L], F32, tag=f"lvl{kk}")
        nc.vector.tensor_tensor(out=nxt[:], in0=a[:][:, :, 0:L],
                                in1=t2[:], op=mybir.AluOpType.add)
        prev = nxt
    wts = prev  # [P, G, 4]

    # ---- weighted sum of the 4 gathered rows, per group ----
    gtv = gt[:].rearrange("p g (d c) -> p g d c", d=WROWS)[:, :, :, 0:C]
    outv = out.rearrange("(g p) c -> p g c", p=P)
    if debug_dump == "i0":
        nc.sync.dma_start(out=outv[:, :, 0], in_=i0f[:])
        return
    if debug_dump == "kw":
        nc.sync.dma_start(out=outv[:, :, 0:WKNOTS], in_=kwv)
        return
    if debug_dump == "wts":
        nc.sync.dma_start(out=outv[:, :, 0:WROWS], in_=wts[:])
        return
    if debug_dump == "gt":
        gf = sb.tile([P, G, C], F32, tag="gf")
        nc.vector.tensor_copy(out=gf[:], in_=gtv[:, :, 0, :])
        nc.sync.dma_start(out=outv[:, :, 0:C], in_=gf[:])
        return
    acc = sb.tile([P, G, C], F32, tag="acc")
    A = act_groups
    GA = G - A  # groups on the DVE STT chain
    for g in range(GA):
        nc.vector.tensor_scalar_mul(out=acc[:][:, g, :],
                                    in0=gtv[:, g, 0, :],
                                    scalar1=wts[:][:, g, 0:1])
        for d in range(1, WROWS):
            nc.vector.scalar_tensor_tensor(
                out=acc[:][:, g, :], in0=gtv[:, g, d, :],
                scalar=wts[:][:, g, d:d + 1], in1=acc[:][:, g, :],
                op0=mybir.AluOpType.mult, op1=mybir.AluOpType.add)
    if A:
        # trailing groups: multiplies on ACT, adds folded across groups on DVE
        prod = sb.tile([P, A, WROWS, C], F32, tag="prod", name="prod")
        for g in range(GA, G):
            for d in range(WROWS):
                nc.scalar.activation(out=prod[:][:, g - GA, d, :],
                                     in_=gtv[:, g, d, :],
                                     func=mybir.ActivationFunctionType.Copy,
                                     scale=wts[:][:, g, d:d + 1])
        pv = prod[:]
        nc.vector.tensor_tensor(out=pv[:, :, 0, :], in0=pv[:, :, 0, :],
                                in1=pv[:, :, 1, :], op=mybir.AluOpType.add)
        nc.vector.tensor_tensor(out=pv[:, :, 2, :], in0=pv[:, :, 2, :],
                                in1=pv[:, :, 3, :], op=mybir.AluOpType.add)
        nc.vector.tensor_tensor(out=acc[:][:, GA:G, :], in0=pv[:, :, 0, :],
                                in1=pv[:, :, 2, :], op=mybir.AluOpType.add)
    nc.scalar.dma_start(out=outv, in_=acc[:])


def _declare(nc, use_kwin=True, coef_bf16=False, combo=False, ktx=False):
    if combo:
        coef = nc.dram_tensor("combo", [BC, N, CROW], F32,
                              kind="ExternalInput")
    else:
        coef = nc.dram_tensor("coefficients", [BC, N, C],
                              BF16 if coef_bf16 else F32,
                              kind="ExternalInput")
    if ktx:
        assert combo
        knots = nc.dram_tensor("ktx", [BC, N - WROWS + 1], F32,
                               kind="ExternalInput")
        inpce = None
    else:
        knots = nc.dram_tensor("knots", [BC, T], F32, kind="ExternalInput")
        inpce = nc.dram_tensor("inpce", [BC, 1], F32, kind="ExternalInput")
    out = nc.dram_tensor("out", [BC, C], F32, kind="ExternalOutput")
    kwin = (nc.dram_tensor("kwin", [BC, NWIN, WELEM], F32,
                           kind="ExternalInput") if use_kwin else None)
    return coef, knots, inpce, out, kwin


def build_nc(reps=1, bufs=2, scratch=16384, **flags):
    cfg = {**DEFAULTS, **flags}
    nc = bacc.Bacc("TRN2", target_bir_lowering=False, debug=False,
                   num_devices=NCORES, dynamic_dma_scratch_size=scratch,
                   num_swdge_queues=max(cfg["nq"], 1))
    coef, knots, inpce, out, kwin = _declare(nc, cfg["use_kwin"],
                                             cfg["coef_bf16"], cfg["combo"],
                                             cfg["ktx"])
    with tile.TileContext(nc) as tc:
        with tc.tile_pool(name="hoist", bufs=1) as hp, \
             tc.tile_pool(name="sb", bufs=bufs) as sb:
            hoisted = _emit_hoisted(tc, nc, hp, cfg["use_kwin"], cfg["use_dma_gather"])
            for _ in range(reps):
                _emit(tc, nc, sb, hoisted, coef.ap(), knots.ap(),
                      inpce.ap() if inpce else None, out.ap(),
                      kwin=kwin.ap() if kwin else None, **cfg)
    nc.compile()
    return nc


def build_nc_loop(trip, unroll=16, bufs=3, scratch=16384, **flags):
    cfg = {**DEFAULTS, **flags}
    nc = bacc.Bacc("TRN2", target_bir_lowering=False, debug=False,
                   num_devices=NCORES, dynamic_dma_scratch_size=scratch,
                   num_swdge_queues=max(cfg["nq"], 1))
    coef, knots, inpce, out, kwin = _declare(nc, cfg["use_kwin"],
                                             cfg["coef_bf16"], cfg["combo"],
                                             cfg["ktx"])
    with tile.TileContext(nc) as tc:
        with tc.tile_pool(name="hoist", bufs=1) as hp, \
             tc.tile_pool(name="sb", bufs=bufs) as sb:
            hoisted = _emit_hoisted(tc, nc, hp, cfg["use_kwin"], cfg["use_dma_gather"])
            with tc.For_i(0, trip, 1):
                for _ in range(unroll):
                    _emit(tc, nc, sb, hoisted, coef.ap(), knots.ap(),
                          inpce.ap() if inpce else None, out.ap(),
                          kwin=kwin.ap() if kwin else None, **cfg)
    nc.compile()
    return nc


def host_kwin(knots):
    """[B', T] -> [B', NWIN, WELEM]: kwin[b, m, :] = knots[b, m:m+64] padded.

    Value-independent input marshalling: row m is the 64-float slab starting
    at knot m, so the device can gather window t[i0..] as one 256B element."""
    pad = NWIN - 1 + WELEM - knots.shape[1]
    kp = np.pad(knots, ((0, 0), (0, pad)), constant_values=2.0)
    w = np.lib.stride_tricks.sliding_window_view(kp, WELEM, axis=1)[:, :NWIN]
    return np.ascontiguousarray(w, dtype=np.float32)


def host_combo(coefficients, knots):
    """[B', N, C] + [B', T] -> [B', N, CROW]: row i = [coef[:, i, :],
    knots[:, i:i+8]] so the coef gather carries the knot window (row i0's
    window is t[i0..i0+7]). Value-independent layout marshalling."""
    Bp = coefficients.shape[0]
    kp = np.pad(np.asarray(knots, dtype=np.float32),
                ((0, 0), (0, N + WKNOTS - 1 - T)), constant_values=2.0)
    sw = np.lib.stride_tricks.sliding_window_view(kp, WKNOTS, axis=1)[:, :N]
    cb = np.empty((Bp, N, CROW), dtype=np.float32)
    cb[:, :, 0:C] = coefficients
    cb[:, :, C:CROW] = sw
    return cb


def host_inputs(coefficients, knots, inpce, use_kwin=None, coef_bf16=None,
                combo=None, ktx=None):
    """Full-size input dict -> per-name arrays incl. host-marshalled extras."""
    if use_kwin is None:
        use_kwin = DEFAULTS["use_kwin"]
    if coef_bf16 is None:
        coef_bf16 = DEFAULTS["coef_bf16"]
    if combo is None:
        combo = DEFAULTS["combo"]
    if ktx is None:
        ktx = DEFAULTS["ktx"]
    if ktx:
        kx = np.concatenate(
            [np.asarray(knots, dtype=np.float32)[:, WROWS:N],
             np.asarray(inpce, dtype=np.float32)], axis=1)
        d = {"ktx": np.ascontiguousarray(kx)}
    else:
        d = {"knots": np.ascontiguousarray(knots, dtype=np.float32),
             "inpce": np.ascontiguousarray(inpce, dtype=np.float32)}
    kn32 = np.ascontiguousarray(knots, dtype=np.float32)
    if combo:
        d["combo"] = host_combo(np.ascontiguousarray(coefficients,
                                                     dtype=np.float32),
                                kn32)
    else:
        cdt = mybir.dt.np(BF16) if coef_bf16 else np.float32
        d["coefficients"] = np.ascontiguousarray(coefficients).astype(cdt)
    if use_kwin:
        d["kwin"] = host_kwin(kn32)
    return d


_NC_CACHE = None


def kernel(coefficients, knots, inpce, **run_kwargs):
    global _NC_CACHE
    if _NC_CACHE is None:
        _NC_CACHE = build_nc()
    nc = _NC_CACHE
    full = host_inputs(coefficients, knots, inpce)
    in_maps = []
    for k in range(NCORES):
        s = slice(k * BC, (k + 1) * BC)
        in_maps.append({name: arr[s] for name, arr in full.items()})
    res = run_bass_kernel_spmd(nc, in_maps, core_ids=list(range(NCORES)),
                               **run_kwargs)
    out = np.concatenate([res.results[k]["out"] for k in range(NCORES)], axis=0)
    if run_kwargs:
        return out, res
    return out
